# revision 23
# baseline (speedup 1.0000x reference)
"""Expert-parallel MoE FFN kernel for Trainium2 (8 NeuronCores).

Strategy (per spec sharding_hint): the router (xf @ router_w, argmax) is the
sharding function -- computed on host (0.05% of total FLOPs), tokens are
dispatched to the core owning their expert (1 expert/core), each core runs a
dense two-layer FFN (D=1024 -> F=4096 -> D=1024) over its token batch with
the expert weight matrices resident in SBUF (bf16), fp32 PSUM accumulation.
Host scatters per-core outputs back to token order.

Load balancing: per-core capacity is 1920 "main" tokens (own expert, weights
resident) plus one 256-token "extra" slot that can host another expert's
overflow; the extra slot's weights are streamed into the same SBUF tiles
after the last main chunk releases them.  16384 tokens / 8 cores = 2048 mean,
so capacity 2176 covers the binomial spread of top-1 routing; a host-side
fallback handles any residual overflow exactly.

All DRAM inputs are host-swizzled into SBUF layout (partition-major, one
contiguous run per partition per transfer) so every DMA is a cheap 2D
descriptor set -- 3D access patterns cost ~5-9us of serial issue time on the
sync sequencer and were the dominant lead-in cost.

Device layout (no transposes on device):
  layer1: psum_hT[f128, tok] += w1[d128, f128].T @ xT[d128, tok]   (lhsT = w1 tile)
          hT = relu(psum + b1)                                     (bias per-partition)
  layer2: psum_y[tok128, d512] += hT[f128, tok128].T @ w2[f128, d512]  (lhsT = hT tile)
"""

import numpy as np
import ml_dtypes

B, S, D, E = 8, 2048, 1024, 8
F = 4 * D
T = B * S
NCORES = 8
PART = 128
NF = 512            # moving-operand free dim per matmul
KD = D // PART      # 8 k-tiles over D
JF = F // PART      # 32 f-tiles over F
CAPM = 1920         # main-expert token capacity
CAPX = 256          # extra-slot token capacity (one donor expert per core)

_CHUNKS = [512, 512, 512, 384]
assert sum(_CHUNKS) == CAPM
_W1_WAVES = [(0, 256), (256, 1024), (1024, 2560), (2560, 4096)]

_cached = None      # built once per process


def _build_program():
    import concourse.tile as tile
    from concourse import bacc, mybir
    from contextlib import ExitStack

    bf16 = mybir.dt.bfloat16
    f32 = mybir.dt.float32
    Relu = mybir.ActivationFunctionType.Relu

    nc = bacc.Bacc("TRN2", target_bir_lowering=False, debug=False)
    # all inputs pre-swizzled on host: [128 partitions, contiguous free dim]
    xT_d = nc.declare_dram_parameter("xT", [PART, KD * CAPM], bf16, isOutput=False)
    w1_d = nc.declare_dram_parameter("w1", [PART, KD * F], bf16, isOutput=False)
    w2_d = nc.declare_dram_parameter("w2", [PART, JF * D], bf16, isOutput=False)
    b1_d = nc.declare_dram_parameter("b1", [PART, JF], f32, isOutput=False)
    xX_d = nc.declare_dram_parameter("xX", [PART, KD * CAPX], bf16, isOutput=False)
    w1x_d = nc.declare_dram_parameter("w1x", [PART, KD * F], bf16, isOutput=False)
    w2x_d = nc.declare_dram_parameter("w2x", [PART, JF * D], bf16, isOutput=False)
    b1x_d = nc.declare_dram_parameter("b1x", [PART, JF], f32, isOutput=False)
    y_d = nc.declare_dram_parameter("y", [CAPM, D], f32, isOutput=True)
    yx_d = nc.declare_dram_parameter("yx", [CAPX, D], f32, isOutput=True)

    # per-partition element offsets of w1 waves / x chunks in the flat rows
    wave_off = [KD * f0 for f0, _ in _W1_WAVES]
    chunk_off = np.cumsum([0] + [KD * n for n in _CHUNKS]).tolist()

    with tile.TileContext(nc) as tc, ExitStack() as ctx:
        wpool = ctx.enter_context(tc.tile_pool(name="wpool", bufs=1))
        xpool = ctx.enter_context(tc.tile_pool(name="xpool", bufs=2))
        hpool = ctx.enter_context(tc.tile_pool(name="hpool", bufs=1))
        ypool = ctx.enter_context(tc.tile_pool(name="ypool", bufs=2))
        ps1 = ctx.enter_context(tc.tile_pool(name="ps1", bufs=2, space="PSUM"))
        ps2 = ctx.enter_context(tc.tile_pool(name="ps2", bufs=2, space="PSUM"))

        # ---- HAM warm-up: dependency-free dummy matmuls run during the DMA
        # lead-in so the PE clock is at 2.4GHz when the real stream starts
        wdum = wpool.tile([PART, 128], bf16, name="wdum", tag="wdum")
        nc.vector.memset(wdum, 0.0)
        pswarm = ps1.tile([PART, NF], f32, name="pswarm", tag="warm", bufs=1)
        for _ in range(200):
            nc.tensor.matmul(pswarm[:32, :64], lhsT=wdum[:, :32],
                             rhs=wdum[:, 0:64], start=True, stop=True)

        # ---- lead-in: chunk-0 tokens, b1, first w1 wave; all 2D DMAs ----
        xt0 = xpool.tile([PART, KD * _CHUNKS[0]], bf16, name="xt0",
                         tag=f"xt{_CHUNKS[0]}")
        nc.sync.dma_start(out=xt0, in_=xT_d[:, 0:KD * _CHUNKS[0]])
        w1w = [wpool.tile([PART, KD, f1 - f0], bf16, name=f"w1w{i}", tag=f"w1w{i}")
               for i, (f0, f1) in enumerate(_W1_WAVES)]
        nc.sync.dma_start(
            out=w1w[0], in_=w1_d[:, wave_off[0]:wave_off[0] + KD * _W1_WAVES[0][1]]
            .rearrange("p (k f) -> p k f", k=KD))
        b1t = wpool.tile([PART, JF], f32, name="b1t", tag="b1t")
        nc.sync.dma_start(out=b1t, in_=b1_d[:, :])
        b1xt = wpool.tile([PART, JF], f32, name="b1xt", tag="b1xt")
        nc.sync.dma_start(out=b1xt, in_=b1x_d[:, :])
        for i, (f0, f1) in enumerate(_W1_WAVES[1:], start=1):
            nc.sync.dma_start(
                out=w1w[i], in_=w1_d[:, wave_off[i]:wave_off[i] + KD * (f1 - f0)]
                .rearrange("p (k f) -> p k f", k=KD))
        w2t = wpool.tile([PART, JF, D], bf16, name="w2t", tag="w2t")

        def w1slice(jf, kd):
            f = jf * PART
            for i, (f0, f1) in enumerate(_W1_WAVES):
                if f0 <= f < f1:
                    return w1w[i][:, kd, f - f0:f - f0 + PART]
            raise AssertionError

        def layer1(xt, b1tile, ntok, tag):
            hts = []
            for jf in range(JF):
                ps = ps1.tile([PART, NF], f32, name=f"ps1_{tag}_{jf}", tag="ps1")
                for kd in range(KD):
                    nc.tensor.matmul(
                        ps[:, :ntok],
                        lhsT=w1slice(jf, kd),
                        rhs=xt[:, kd * ntok:(kd + 1) * ntok],
                        start=(kd == 0),
                        stop=(kd == KD - 1),
                    )
                ht = hpool.tile([PART, NF], bf16, name=f"ht_{tag}_{jf}", tag=f"ht{jf}")
                nc.scalar.activation(ht[:, :ntok], ps[:, :ntok], Relu,
                                     bias=b1tile[:, jf:jf + 1])
                hts.append(ht)
            return hts

        def layer2(hts, out_d, t0, ntok, tag):
            for it in range(ntok // PART):
                psa = ps2.tile([PART, NF], f32, name=f"ps2a_{tag}_{it}", tag="ps2a")
                psb = ps2.tile([PART, NF], f32, name=f"ps2b_{tag}_{it}", tag="ps2b")
                for jf in range(JF):
                    lhsT = hts[jf][:, it * PART:(it + 1) * PART]
                    nc.tensor.matmul(psa, lhsT=lhsT, rhs=w2t[:, jf, 0:NF],
                                     start=(jf == 0), stop=(jf == JF - 1))
                    nc.tensor.matmul(psb, lhsT=lhsT, rhs=w2t[:, jf, NF:D],
                                     start=(jf == 0), stop=(jf == JF - 1))
                yta = ypool.tile([PART, NF], f32, name=f"yta_{tag}_{it}", tag="yta")
                ytb = ypool.tile([PART, NF], f32, name=f"ytb_{tag}_{it}", tag="ytb")
                nc.vector.tensor_copy(yta, psa)
                nc.vector.tensor_copy(ytb, psb)
                row = t0 + it * PART
                nc.sync.dma_start(out=out_d[row:row + PART, 0:NF], in_=yta)
                nc.sync.dma_start(out=out_d[row:row + PART, NF:D], in_=ytb)

        t0 = 0
        for ci, ntok in enumerate(_CHUNKS):
            if ci == 0:
                xt = xt0
            else:
                xt = xpool.tile([PART, KD * ntok], bf16, name=f"xt{ci}",
                                tag=f"xt{ntok}", bufs=1 if ntok == 384 else None)
                nc.sync.dma_start(
                    out=xt, in_=xT_d[:, chunk_off[ci]:chunk_off[ci] + KD * ntok])
            hts = layer1(xt, b1t, ntok, f"m{ci}")
            if ci == 0:
                # w2 loads issued after chunk-0 layer-1 so they don't delay it
                for q in range(4):
                    nc.sync.dma_start(out=w2t[:, q * 8:(q + 1) * 8, :],
                                      in_=w2_d[:, q * 8 * D:(q + 1) * 8 * D]
                                      .rearrange("p (j d) -> p j d", j=8))
            if ci == len(_CHUNKS) - 1:
                # extra-slot token load + streamed extra weights: w1x reuses
                # the w1 wave tiles once the last main layer-1 is done (WAR)
                xtx = xpool.tile([PART, KD * CAPX], bf16, name="xtx",
                                 tag=f"xt{CAPX}")
                nc.sync.dma_start(out=xtx, in_=xX_d[:, :])
                for i, (f0, f1) in enumerate(_W1_WAVES):
                    nc.sync.dma_start(
                        out=w1w[i],
                        in_=w1x_d[:, wave_off[i]:wave_off[i] + KD * (f1 - f0)]
                        .rearrange("p (k f) -> p k f", k=KD))
            layer2(hts, y_d, t0, ntok, f"m{ci}")
            t0 += ntok

        # ---- extra chunk: donor expert's overflow tokens ----
        hts = layer1(xtx, b1xt, CAPX, "x")
        for q in range(4):   # w2x reuses w2t after main layer-2 consumed it
            nc.sync.dma_start(out=w2t[:, q * 8:(q + 1) * 8, :],
                              in_=w2x_d[:, q * 8 * D:(q + 1) * 8 * D]
                              .rearrange("p (j d) -> p j d", j=8))
        layer2(hts, yx_d, 0, CAPX, "x")

    nc.compile()
    return nc


def _get_program():
    global _cached
    if _cached is None:
        _cached = _build_program()
    return _cached


_BF = ml_dtypes.bfloat16


def _swizzle_w1(w1e):
    """[D, F] -> [128, KD*F] bf16, wave-concat, kd-major f-minor per wave."""
    a = np.ascontiguousarray(w1e).astype(_BF).reshape(KD, PART, F)
    blocks = [a[:, :, f0:f1].transpose(1, 0, 2).reshape(PART, -1)
              for f0, f1 in _W1_WAVES]
    return np.ascontiguousarray(np.concatenate(blocks, axis=1))


def _swizzle_w2(w2e):
    """[F, D] -> [128, JF*D] bf16, jf-major."""
    a = np.ascontiguousarray(w2e).astype(_BF).reshape(JF, PART, D)
    return np.ascontiguousarray(a.transpose(1, 0, 2).reshape(PART, JF * D))


def _swizzle_x(xtok, cap, chunks):
    """[n, D] tokens -> [128, KD*cap] bf16, per chunk kd-major token-minor."""
    xp = np.zeros((cap, D), dtype=np.float32)
    xp[:len(xtok)] = xtok
    out = np.empty((PART, KD * cap), dtype=_BF)
    o = 0
    t0 = 0
    for ntok in chunks:
        blk = xp[t0:t0 + ntok].T.astype(_BF)          # [D, ntok]
        blk = blk.reshape(KD, PART, ntok).transpose(1, 0, 2).reshape(PART, -1)
        out[:, o:o + KD * ntok] = blk
        t0 += ntok
        o += KD * ntok
    return out


def _swizzle_b1(b1e):
    """[F] -> [128, JF] f32 (partition-major)."""
    return np.ascontiguousarray(
        np.asarray(b1e, dtype=np.float32).reshape(JF, PART).T)


def kernel(x, router_w, router_b, w1, b1, w2, b2, _trace=False):
    from concourse.bass_utils import run_bass_kernel_spmd

    x = np.asarray(x, dtype=np.float32)
    router_w = np.asarray(router_w, dtype=np.float32)
    router_b = np.asarray(router_b, dtype=np.float32)
    w1 = np.asarray(w1, dtype=np.float32)
    b1 = np.asarray(b1, dtype=np.float32)
    w2 = np.asarray(w2, dtype=np.float32)
    b2 = np.asarray(b2, dtype=np.float32)

    xf = x.reshape(-1, D)                         # [T, D]
    logits = xf @ router_w + router_b             # [T, E]
    idx = np.argmax(logits, axis=-1)              # [T]

    w1s = [_swizzle_w1(w1[e]) for e in range(E)]
    w2s = [_swizzle_w2(w2[e]) for e in range(E)]
    b1s = [_swizzle_b1(b1[e]) for e in range(E)]

    main_ids = []                                 # per-core main token indices
    pieces = []                                   # (donor_expert, ids) <= CAPX each
    host_ids = []                                 # residual overflow -> host
    for e in range(E):
        ids = np.nonzero(idx == e)[0]
        main_ids.append(ids[:CAPM])
        rest = ids[CAPM:]
        for s in range(0, len(rest), CAPX):
            pieces.append((e, rest[s:s + CAPX]))
    if len(pieces) > NCORES:                      # safety net; never hit for
        host_ids = pieces[NCORES:]                # the fixed problem inputs
        pieces = pieces[:NCORES]

    in_maps = []
    extra_ids = [None] * NCORES
    for c in range(NCORES):
        ids = main_ids[c]
        if c < len(pieces):
            d, pids = pieces[c]
            extra_ids[c] = (d, pids)
            xXe = _swizzle_x(xf[pids], CAPX, [CAPX])
            w1x, w2x, b1x = w1s[d], w2s[d], b1s[d]
        else:
            xXe = np.zeros((PART, KD * CAPX), dtype=_BF)
            w1x, w2x, b1x = w1s[c], w2s[c], b1s[c]
        in_maps.append({
            "xT": _swizzle_x(xf[ids], CAPM, _CHUNKS),
            "w1": w1s[c],
            "w2": w2s[c],
            "b1": b1s[c],
            "xX": xXe,
            "w1x": w1x,
            "w2x": w2x,
            "b1x": b1x,
        })

    nc = _get_program()
    res = run_bass_kernel_spmd(nc, in_maps, list(range(NCORES)), trace=_trace)

    out = np.zeros((T, D), dtype=np.float32)
    for c in range(NCORES):
        ids = main_ids[c]
        out[ids] = res.results[c]["y"][:len(ids)] + b2[c]
        if extra_ids[c] is not None:
            d, pids = extra_ids[c]
            out[pids] = res.results[c]["yx"][:len(pids)] + b2[d]
    for e, pids in host_ids:                      # host fallback (normally empty)
        h = np.maximum(xf[pids] @ w1[e] + b1[e], 0.0)
        out[pids] = h @ w2[e] + b2[e]

    counts = np.bincount(idx, minlength=E).astype(np.float32)
    usage = counts / counts.sum()
    lb_loss = np.float32(np.mean((usage - np.float32(1.0 / E)) ** 2, dtype=np.float32))

    if _trace:
        return out.reshape(x.shape), lb_loss, res
    return out.reshape(x.shape), lb_loss


# revision 24
# speedup vs baseline: 1.0112x; 1.0112x over previous
"""Expert-parallel MoE FFN kernel for Trainium2 (8 NeuronCores).

Strategy (per spec sharding_hint): the router (xf @ router_w, argmax) is the
sharding function -- computed on host (0.05% of total FLOPs), tokens are
dispatched to the core owning their expert (1 expert/core), each core runs a
dense two-layer FFN (D=1024 -> F=4096 -> D=1024) over its token batch with
the expert weight matrices resident in SBUF (bf16), fp32 PSUM accumulation.
Host scatters per-core outputs back to token order.

Load balancing: per-core capacity is 1920 "main" tokens (own expert, weights
resident) plus one 256-token "extra" slot that can host another expert's
overflow; the extra slot's weights are streamed into the same SBUF tiles
after the last main chunk releases them.  16384 tokens / 8 cores = 2048 mean,
so capacity 2176 covers the binomial spread of top-1 routing; a host-side
fallback handles any residual overflow exactly.

All DRAM inputs are host-swizzled into SBUF layout (partition-major, one
contiguous run per partition per transfer) so every DMA is a cheap 2D
descriptor set -- 3D access patterns cost ~5-9us of serial issue time on the
sync sequencer and were the dominant lead-in cost.

Device layout (no transposes on device):
  layer1: psum_hT[f128, tok] += w1[d128, f128].T @ xT[d128, tok]   (lhsT = w1 tile)
          hT = relu(psum + b1)                                     (bias per-partition)
  layer2: psum_y[tok128, d512] += hT[f128, tok128].T @ w2[f128, d512]  (lhsT = hT tile)
"""

import numpy as np
import ml_dtypes

B, S, D, E = 8, 2048, 1024, 8
F = 4 * D
T = B * S
NCORES = 8
PART = 128
NF = 512            # moving-operand free dim per matmul
KD = D // PART      # 8 k-tiles over D
JF = F // PART      # 32 f-tiles over F
CAPM = 1920         # main-expert token capacity
CAPX = 256          # extra-slot token capacity (one donor expert per core)

_CHUNKS = [512, 512, 512, 384]
assert sum(_CHUNKS) == CAPM
_W1_WAVES = [(0, 256), (256, 1024), (1024, 2560), (2560, 4096)]

_cached = None      # built once per process


def _build_program():
    import concourse.tile as tile
    from concourse import bacc, mybir
    from contextlib import ExitStack

    bf16 = mybir.dt.bfloat16
    f32 = mybir.dt.float32
    Relu = mybir.ActivationFunctionType.Relu

    nc = bacc.Bacc("TRN2", target_bir_lowering=False, debug=False)
    # all inputs pre-swizzled on host: [128 partitions, contiguous free dim]
    xT_d = nc.declare_dram_parameter("xT", [PART, KD * CAPM], bf16, isOutput=False)
    w1_d = nc.declare_dram_parameter("w1", [PART, KD * F], bf16, isOutput=False)
    w2_d = nc.declare_dram_parameter("w2", [PART, JF * D], bf16, isOutput=False)
    b1_d = nc.declare_dram_parameter("b1", [PART, JF], f32, isOutput=False)
    xX_d = nc.declare_dram_parameter("xX", [PART, KD * CAPX], bf16, isOutput=False)
    w1x_d = nc.declare_dram_parameter("w1x", [PART, KD * F], bf16, isOutput=False)
    w2x_d = nc.declare_dram_parameter("w2x", [PART, JF * D], bf16, isOutput=False)
    b1x_d = nc.declare_dram_parameter("b1x", [PART, JF], f32, isOutput=False)
    y_d = nc.declare_dram_parameter("y", [CAPM, D], f32, isOutput=True)
    yx_d = nc.declare_dram_parameter("yx", [CAPX, D], f32, isOutput=True)

    # per-partition element offsets of w1 waves / x chunks in the flat rows
    wave_off = [KD * f0 for f0, _ in _W1_WAVES]
    chunk_off = np.cumsum([0] + [KD * n for n in _CHUNKS]).tolist()

    with tile.TileContext(nc) as tc, ExitStack() as ctx:
        wpool = ctx.enter_context(tc.tile_pool(name="wpool", bufs=1))
        xpool = ctx.enter_context(tc.tile_pool(name="xpool", bufs=2))
        hpool = ctx.enter_context(tc.tile_pool(name="hpool", bufs=1))
        ypool = ctx.enter_context(tc.tile_pool(name="ypool", bufs=2))
        ps1 = ctx.enter_context(tc.tile_pool(name="ps1", bufs=2, space="PSUM"))
        ps2 = ctx.enter_context(tc.tile_pool(name="ps2", bufs=2, space="PSUM"))

        # ---- HAM warm-up: dependency-free dummy matmuls run during the DMA
        # lead-in so the PE clock is at 2.4GHz when the real stream starts
        wdum = wpool.tile([PART, 128], bf16, name="wdum", tag="wdum")
        nc.vector.memset(wdum, 0.0)
        pswarm = ps1.tile([PART, NF], f32, name="pswarm", tag="warm", bufs=1)
        for _ in range(120):
            nc.tensor.matmul(pswarm[:32, :64], lhsT=wdum[:, :32],
                             rhs=wdum[:, 0:64], start=True, stop=True)

        # ---- lead-in: chunk-0 tokens, b1, first w1 wave; all 2D DMAs ----
        xt0 = xpool.tile([PART, KD * _CHUNKS[0]], bf16, name="xt0",
                         tag=f"xt{_CHUNKS[0]}")
        nc.sync.dma_start(out=xt0, in_=xT_d[:, 0:KD * _CHUNKS[0]])
        w1w = [wpool.tile([PART, KD, f1 - f0], bf16, name=f"w1w{i}", tag=f"w1w{i}")
               for i, (f0, f1) in enumerate(_W1_WAVES)]
        nc.sync.dma_start(
            out=w1w[0], in_=w1_d[:, wave_off[0]:wave_off[0] + KD * _W1_WAVES[0][1]]
            .rearrange("p (k f) -> p k f", k=KD))
        b1t = wpool.tile([PART, JF], f32, name="b1t", tag="b1t")
        nc.sync.dma_start(out=b1t, in_=b1_d[:, :])
        b1xt = wpool.tile([PART, JF], f32, name="b1xt", tag="b1xt")
        nc.sync.dma_start(out=b1xt, in_=b1x_d[:, :])
        for i, (f0, f1) in enumerate(_W1_WAVES[1:], start=1):
            nc.sync.dma_start(
                out=w1w[i], in_=w1_d[:, wave_off[i]:wave_off[i] + KD * (f1 - f0)]
                .rearrange("p (k f) -> p k f", k=KD))
        w2t = wpool.tile([PART, JF, D], bf16, name="w2t", tag="w2t")

        def w1slice(jf, kd):
            f = jf * PART
            for i, (f0, f1) in enumerate(_W1_WAVES):
                if f0 <= f < f1:
                    return w1w[i][:, kd, f - f0:f - f0 + PART]
            raise AssertionError

        def layer1(xt, b1tile, ntok, tag):
            hts = []
            for jf in range(JF):
                ps = ps1.tile([PART, NF], f32, name=f"ps1_{tag}_{jf}", tag="ps1")
                for kd in range(KD):
                    nc.tensor.matmul(
                        ps[:, :ntok],
                        lhsT=w1slice(jf, kd),
                        rhs=xt[:, kd * ntok:(kd + 1) * ntok],
                        start=(kd == 0),
                        stop=(kd == KD - 1),
                    )
                ht = hpool.tile([PART, NF], bf16, name=f"ht_{tag}_{jf}", tag=f"ht{jf}")
                nc.scalar.activation(ht[:, :ntok], ps[:, :ntok], Relu,
                                     bias=b1tile[:, jf:jf + 1])
                hts.append(ht)
            return hts

        def layer2(hts, out_d, t0, ntok, tag):
            for it in range(ntok // PART):
                psa = ps2.tile([PART, NF], f32, name=f"ps2a_{tag}_{it}", tag="ps2a")
                psb = ps2.tile([PART, NF], f32, name=f"ps2b_{tag}_{it}", tag="ps2b")
                for jf in range(JF):
                    lhsT = hts[jf][:, it * PART:(it + 1) * PART]
                    nc.tensor.matmul(psa, lhsT=lhsT, rhs=w2t[:, jf, 0:NF],
                                     start=(jf == 0), stop=(jf == JF - 1))
                    nc.tensor.matmul(psb, lhsT=lhsT, rhs=w2t[:, jf, NF:D],
                                     start=(jf == 0), stop=(jf == JF - 1))
                yta = ypool.tile([PART, NF], f32, name=f"yta_{tag}_{it}", tag="yta")
                ytb = ypool.tile([PART, NF], f32, name=f"ytb_{tag}_{it}", tag="ytb")
                nc.vector.tensor_copy(yta, psa)
                nc.vector.tensor_copy(ytb, psb)
                row = t0 + it * PART
                nc.sync.dma_start(out=out_d[row:row + PART, 0:NF], in_=yta)
                nc.sync.dma_start(out=out_d[row:row + PART, NF:D], in_=ytb)

        t0 = 0
        for ci, ntok in enumerate(_CHUNKS):
            if ci == 0:
                xt = xt0
            else:
                xt = xpool.tile([PART, KD * ntok], bf16, name=f"xt{ci}",
                                tag=f"xt{ntok}", bufs=1 if ntok == 384 else None)
                nc.sync.dma_start(
                    out=xt, in_=xT_d[:, chunk_off[ci]:chunk_off[ci] + KD * ntok])
            hts = layer1(xt, b1t, ntok, f"m{ci}")
            if ci == 0:
                # w2 loads issued after chunk-0 layer-1 so they don't delay it
                for q in range(4):
                    nc.sync.dma_start(out=w2t[:, q * 8:(q + 1) * 8, :],
                                      in_=w2_d[:, q * 8 * D:(q + 1) * 8 * D]
                                      .rearrange("p (j d) -> p j d", j=8))
            if ci == len(_CHUNKS) - 1:
                # extra-slot token load + streamed extra weights: w1x reuses
                # the w1 wave tiles once the last main layer-1 is done (WAR)
                xtx = xpool.tile([PART, KD * CAPX], bf16, name="xtx",
                                 tag=f"xt{CAPX}")
                nc.sync.dma_start(out=xtx, in_=xX_d[:, :])
                for i, (f0, f1) in enumerate(_W1_WAVES):
                    nc.sync.dma_start(
                        out=w1w[i],
                        in_=w1x_d[:, wave_off[i]:wave_off[i] + KD * (f1 - f0)]
                        .rearrange("p (k f) -> p k f", k=KD))
            layer2(hts, y_d, t0, ntok, f"m{ci}")
            t0 += ntok

        # ---- extra chunk: donor expert's overflow tokens ----
        hts = layer1(xtx, b1xt, CAPX, "x")
        for q in range(4):   # w2x reuses w2t after main layer-2 consumed it
            nc.sync.dma_start(out=w2t[:, q * 8:(q + 1) * 8, :],
                              in_=w2x_d[:, q * 8 * D:(q + 1) * 8 * D]
                              .rearrange("p (j d) -> p j d", j=8))
        layer2(hts, yx_d, 0, CAPX, "x")

    nc.compile()
    return nc


def _get_program():
    global _cached
    if _cached is None:
        _cached = _build_program()
    return _cached


_BF = ml_dtypes.bfloat16


def _swizzle_w1(w1e):
    """[D, F] -> [128, KD*F] bf16, wave-concat, kd-major f-minor per wave."""
    a = np.ascontiguousarray(w1e).astype(_BF).reshape(KD, PART, F)
    blocks = [a[:, :, f0:f1].transpose(1, 0, 2).reshape(PART, -1)
              for f0, f1 in _W1_WAVES]
    return np.ascontiguousarray(np.concatenate(blocks, axis=1))


def _swizzle_w2(w2e):
    """[F, D] -> [128, JF*D] bf16, jf-major."""
    a = np.ascontiguousarray(w2e).astype(_BF).reshape(JF, PART, D)
    return np.ascontiguousarray(a.transpose(1, 0, 2).reshape(PART, JF * D))


def _swizzle_x(xtok, cap, chunks):
    """[n, D] tokens -> [128, KD*cap] bf16, per chunk kd-major token-minor."""
    xp = np.zeros((cap, D), dtype=np.float32)
    xp[:len(xtok)] = xtok
    out = np.empty((PART, KD * cap), dtype=_BF)
    o = 0
    t0 = 0
    for ntok in chunks:
        blk = xp[t0:t0 + ntok].T.astype(_BF)          # [D, ntok]
        blk = blk.reshape(KD, PART, ntok).transpose(1, 0, 2).reshape(PART, -1)
        out[:, o:o + KD * ntok] = blk
        t0 += ntok
        o += KD * ntok
    return out


def _swizzle_b1(b1e):
    """[F] -> [128, JF] f32 (partition-major)."""
    return np.ascontiguousarray(
        np.asarray(b1e, dtype=np.float32).reshape(JF, PART).T)


def kernel(x, router_w, router_b, w1, b1, w2, b2, _trace=False):
    from concourse.bass_utils import run_bass_kernel_spmd

    x = np.asarray(x, dtype=np.float32)
    router_w = np.asarray(router_w, dtype=np.float32)
    router_b = np.asarray(router_b, dtype=np.float32)
    w1 = np.asarray(w1, dtype=np.float32)
    b1 = np.asarray(b1, dtype=np.float32)
    w2 = np.asarray(w2, dtype=np.float32)
    b2 = np.asarray(b2, dtype=np.float32)

    xf = x.reshape(-1, D)                         # [T, D]
    logits = xf @ router_w + router_b             # [T, E]
    idx = np.argmax(logits, axis=-1)              # [T]

    w1s = [_swizzle_w1(w1[e]) for e in range(E)]
    w2s = [_swizzle_w2(w2[e]) for e in range(E)]
    b1s = [_swizzle_b1(b1[e]) for e in range(E)]

    main_ids = []                                 # per-core main token indices
    pieces = []                                   # (donor_expert, ids) <= CAPX each
    host_ids = []                                 # residual overflow -> host
    for e in range(E):
        ids = np.nonzero(idx == e)[0]
        main_ids.append(ids[:CAPM])
        rest = ids[CAPM:]
        for s in range(0, len(rest), CAPX):
            pieces.append((e, rest[s:s + CAPX]))
    if len(pieces) > NCORES:                      # safety net; never hit for
        host_ids = pieces[NCORES:]                # the fixed problem inputs
        pieces = pieces[:NCORES]

    in_maps = []
    extra_ids = [None] * NCORES
    for c in range(NCORES):
        ids = main_ids[c]
        if c < len(pieces):
            d, pids = pieces[c]
            extra_ids[c] = (d, pids)
            xXe = _swizzle_x(xf[pids], CAPX, [CAPX])
            w1x, w2x, b1x = w1s[d], w2s[d], b1s[d]
        else:
            xXe = np.zeros((PART, KD * CAPX), dtype=_BF)
            w1x, w2x, b1x = w1s[c], w2s[c], b1s[c]
        in_maps.append({
            "xT": _swizzle_x(xf[ids], CAPM, _CHUNKS),
            "w1": w1s[c],
            "w2": w2s[c],
            "b1": b1s[c],
            "xX": xXe,
            "w1x": w1x,
            "w2x": w2x,
            "b1x": b1x,
        })

    nc = _get_program()
    res = run_bass_kernel_spmd(nc, in_maps, list(range(NCORES)), trace=_trace)

    out = np.zeros((T, D), dtype=np.float32)
    for c in range(NCORES):
        ids = main_ids[c]
        out[ids] = res.results[c]["y"][:len(ids)] + b2[c]
        if extra_ids[c] is not None:
            d, pids = extra_ids[c]
            out[pids] = res.results[c]["yx"][:len(pids)] + b2[d]
    for e, pids in host_ids:                      # host fallback (normally empty)
        h = np.maximum(xf[pids] @ w1[e] + b1[e], 0.0)
        out[pids] = h @ w2[e] + b2[e]

    counts = np.bincount(idx, minlength=E).astype(np.float32)
    usage = counts / counts.sum()
    lb_loss = np.float32(np.mean((usage - np.float32(1.0 / E)) ** 2, dtype=np.float32))

    if _trace:
        return out.reshape(x.shape), lb_loss, res
    return out.reshape(x.shape), lb_loss
